# revision 9
# baseline (speedup 1.0000x reference)
"""Trainium2 Bass kernel for nn_DecoderRNN: GRU decoder (teacher forcing) +
vocab projection + log_softmax.

Strategy: data-parallel over batch across 8 NeuronCores (8 batch rows/core).
Per core (all in "transposed" layouts, feature dims on SBUF partitions):
  - gi = relu(emb) @ W_ih.T + bias  precomputed for all 30 steps (one matmul)
  - 30-step GRU recurrence: ghT via weight-stationary bf16 matmuls,
    gates on [128, .] tiles, sigmoid computed via tanh (one ACT table set)
  - vocab projection (hs stationary, W_out.T streamed, b_out via K=1 matmul),
    interleaved into recurrence gaps once row-tile 0 is complete
  - two-pass log_softmax: per-block max/sum-exp stats during projection,
    bf16 logits round-trip through HBM, final per-partition lse subtract
"""

from contextlib import ExitStack, nullcontext

import numpy as np
import ml_dtypes

import concourse.bass as bass
import concourse.mybir as mybir
import concourse.tile as tile
from concourse import bacc
from concourse.bass_utils import run_bass_kernel_spmd

BF16 = mybir.dt.bfloat16
F32 = mybir.dt.float32
AF = mybir.ActivationFunctionType
ALU = mybir.AluOpType
AX = mybir.AxisListType

# problem dims (hardcoded per harness contract)
B = 64
T = 30
H = 1024
V = 32000
NCORES = 8
BL = B // NCORES          # batch rows per core
SOS = 1


def _blocks(total, step):
    out = []
    off = 0
    while off < total:
        sz = min(step, total - off)
        out.append((off, sz))
        off += sz
    return out


def build_nc(t_steps=T, vocab=V, h=H, bl=BL, num_devices=NCORES, nrep=1,
             mode="all"):
    """Build the per-core bass program (same program on all cores, SPMD).

    mode: 'all' | 'gru' (loads+gi+recurrence only) | 'proj' (projection only)
    nrep: repeat whole body in a hardware For_i loop (for wall-delta timing)
    """
    do_gru = mode in ("all", "gru", "all_noint")
    do_proj = mode in ("all", "proj", "all_noint")
    KC = h // 128             # H chunks on partitions
    G3 = 3 * h
    MT = G3 // 128            # gate-dim tiles (r, z, n each KC tiles)
    R = t_steps * bl          # rows per core; row index = t*bl + b
    vblocks = _blocks(vocab, 512)
    mtiles = _blocks(R, 128)
    NBV = len(vblocks)
    NMT = len(mtiles)
    mtile_ready = [(off + pm - 1) // bl for off, pm in mtiles]

    nc = bacc.Bacc("TRN2", target_bir_lowering=False, debug=False,
                   num_devices=num_devices)

    # ---- DRAM I/O ----
    xT_d = nc.dram_tensor("xT", [h, R], BF16, kind="ExternalInput")
    h0_d = nc.dram_tensor("h0T", [128, KC * bl], F32, kind="ExternalInput")
    wih_d = nc.dram_tensor("wihT", [h, G3], BF16, kind="ExternalInput")
    whh_d = nc.dram_tensor("whhT", [h, G3], BF16, kind="ExternalInput")
    bgi_d = nc.dram_tensor("bgi", [G3], F32, kind="ExternalInput")
    bhn_d = nc.dram_tensor("bhn", [128, KC * bl], F32, kind="ExternalInput")
    wout_d = nc.dram_tensor("woutT", [h, vocab], BF16, kind="ExternalInput")
    bout_d = nc.dram_tensor("bout", [vocab], BF16, kind="ExternalInput")
    lp_d = nc.dram_tensor("log_probs", [R, vocab], F32, kind="ExternalOutput")
    hf_d = nc.dram_tensor("h_final", [128, KC * bl], F32, kind="ExternalOutput")
    scr_d = nc.dram_tensor("scratch", [R, vocab], BF16)  # internal HBM scratch

    with tile.TileContext(nc) as tc, ExitStack() as ctx:
        const = ctx.enter_context(tc.tile_pool(name="const", bufs=1))
        work = ctx.enter_context(tc.tile_pool(name="work", bufs=3))
        gpsum = ctx.enter_context(tc.tile_pool(name="gpsum", bufs=2, space="PSUM"))

        loop_cm = tc.For_i(0, nrep, 1) if nrep > 1 else nullcontext()
        with loop_cm:
            # ---- resident SBUF tiles ----
            whh = const.tile([128, KC, G3], BF16)
            xT = const.tile([128, KC, R], BF16)
            giT = const.tile([128, MT, R], F32)
            hs_m = [const.tile([128, KC, pm], BF16, name=f"hs{i}", tag=f"hs{i}")
                    for i, (off, pm) in enumerate(mtiles)]
            h0b = const.tile([128, KC * bl], BF16)
            bgi = const.tile([128, MT], F32)
            bhn = const.tile([128, KC * bl], F32)
            ones = const.tile([1, 128], BF16)
            nbm = [const.tile([128, NBV], F32, name=f"nbm{i}", tag=f"nbm{i}")
                   for i in range(NMT)]
            bsum = [const.tile([128, NBV], F32, name=f"bsum{i}", tag=f"bsum{i}")
                    for i in range(NMT)]
            nlse = [const.tile([128, 1], F32, name=f"nlse{i}", tag=f"nlse{i}")
                    for i in range(NMT)]

            # ---- load ----
            nc.vector.memset(ones, 1.0)
            h_prev_f32 = const.tile([128, KC * bl], F32, tag="h0f", name="h0f")
            nc.sync.dma_start(out=h_prev_f32, in_=h0_d.ap())
            if do_gru:
                nc.sync.dma_start(out=whh, in_=whh_d.ap().rearrange("(k p) m -> p k m", p=128))
                nc.sync.dma_start(out=xT, in_=xT_d.ap().rearrange("(k p) r -> p k r", p=128))
                nc.sync.dma_start(out=bgi, in_=bgi_d.ap().rearrange("(m p) -> p m", p=128))
                nc.sync.dma_start(out=bhn, in_=bhn_d.ap())
                nc.vector.tensor_relu(xT, xT)
                nc.vector.tensor_copy(h0b, h_prev_f32)
            else:
                nc.vector.memset(giT, 0.01)
                for mi in range(NMT):
                    nc.vector.memset(hs_m[mi], 0.01)

            # ---- phase 1: giT = W_ih @ x + bias (transposed layout) ----
            if do_gru:
                with (
                    tc.tile_pool(name="wihp", bufs=1) as wihp,
                    tc.tile_pool(name="gipsum", bufs=3, space="PSUM") as gipsum,
                ):
                    wih = wihp.tile([128, KC, G3], BF16)
                    nc.sync.dma_start(out=wih, in_=wih_d.ap().rearrange("(k p) m -> p k m", p=128))
                    for m in range(MT):
                        pg = gipsum.tile([128, R], F32, tag="pg", name="pg")
                        for k in range(KC):
                            nc.tensor.matmul(pg, wih[:, k, m * 128:(m + 1) * 128],
                                             xT[:, k, :], start=(k == 0),
                                             stop=(k == KC - 1))
                        nc.vector.tensor_scalar(giT[:, m, :], pg, bgi[:, m:m + 1],
                                                None, op0=ALU.add)

            # ---- projection / softmax helpers ----
            wpool = ctx.enter_context(tc.tile_pool(name="wstage", bufs=4))
            lgpool = ctx.enter_context(tc.tile_pool(name="lg", bufs=4))
            outp = ctx.enter_context(tc.tile_pool(name="outp", bufs=4))
            ppsum = ctx.enter_context(tc.tile_pool(name="ppsum", bufs=3, space="PSUM"))

            def stage_w(vb):
                voff, vn = vblocks[vb]
                wt = wpool.tile([128, KC, 512], BF16, tag="wt", name="wt")
                nc.sync.dma_start(
                    out=wt[:, :, :vn],
                    in_=wout_d.ap()[:, voff:voff + vn].rearrange("(k p) v -> p k v", p=128))
                bt = wpool.tile([1, 512], BF16, tag="bt", name="bt")
                nc.sync.dma_start(out=bt[:, :vn],
                                  in_=bout_d.ap()[voff:voff + vn].rearrange("(a v) -> a v", a=1))
                return wt, bt

            def proj_block(mi, vb, wt, bt):
                moff, pm = mtiles[mi]
                voff, vn = vblocks[vb]
                pl = ppsum.tile([128, 512], F32, tag="pl", name="pl")
                nc.tensor.matmul(pl[:pm, :vn], ones[:, :pm], bt[:, :vn],
                                 start=True, stop=False)
                for k in range(KC):
                    nc.tensor.matmul(pl[:pm, :vn], hs_m[mi][:, k, :pm], wt[:, k, :vn],
                                     start=False, stop=(k == KC - 1))
                nc.vector.tensor_reduce(nbm[mi][:pm, vb:vb + 1], pl[:pm, :vn],
                                        axis=AX.X, op=ALU.max, negate=True)
                lg = lgpool.tile([128, 512], BF16, tag="lg", name="lg")
                nc.scalar.copy(lg[:pm, :vn], pl[:pm, :vn])
                nc.sync.dma_start(out=scr_d.ap()[moff:moff + pm, voff:voff + vn],
                                  in_=lg[:pm, :vn])
                et = lgpool.tile([128, 512], BF16, tag="et", name="et")
                nc.scalar.activation(et[:pm, :vn], pl[:pm, :vn], AF.Exp,
                                     bias=nbm[mi][:pm, vb:vb + 1],
                                     accum_out=bsum[mi][:pm, vb:vb + 1])

            def stats(mi):
                moff, pm = mtiles[mi]
                negM = work.tile([128, 1], F32, tag="negM", name="negM")
                nc.vector.tensor_reduce(negM[:pm], nbm[mi][:pm, :], axis=AX.X,
                                        op=ALU.min)
                corr = work.tile([128, NBV], F32, tag="corr", name="corr")
                nc.scalar.activation(corr[:pm], nbm[mi][:pm, :], AF.Exp,
                                     bias=negM[:pm], scale=-1.0)
                prod = work.tile([128, NBV], F32, tag="prod", name="prod")
                nc.vector.tensor_mul(prod[:pm], corr[:pm], bsum[mi][:pm, :])
                S = work.tile([128, 1], F32, tag="S", name="S")
                nc.vector.tensor_reduce(S[:pm], prod[:pm], axis=AX.X, op=ALU.add)
                lnS = work.tile([128, 1], F32, tag="lnS", name="lnS")
                nc.scalar.activation(lnS[:pm], S[:pm], AF.Ln)
                nc.vector.tensor_tensor(nlse[mi][:pm], negM[:pm], lnS[:pm],
                                        op=ALU.subtract)

            def pass_c(mi, vb):
                moff, pm = mtiles[mi]
                voff, vn = vblocks[vb]
                lr = lgpool.tile([128, 512], BF16, tag="lr", name="lr")
                nc.sync.dma_start(out=lr[:pm, :vn],
                                  in_=scr_d.ap()[moff:moff + pm, voff:voff + vn])
                of = outp.tile([128, 512], F32, tag="of", name="of")
                nc.vector.tensor_scalar(of[:pm, :vn], lr[:pm, :vn],
                                        nlse[mi][:pm], None, op0=ALU.add)
                nc.sync.dma_start(out=lp_d.ap()[moff:moff + pm, voff:voff + vn],
                                  in_=of[:pm, :vn])

            # schedule: row-tile 0 blocks spread over steps after it completes
            sched = {t: [] for t in range(t_steps)}
            early_done = 0
            if (mode != "all_noint") and do_proj and do_gru and NMT > 1 and mtile_ready[0] < t_steps - 1:
                avail = list(range(mtile_ready[0] + 1, t_steps))
                per = (NBV + len(avail) - 1) // len(avail)
                i = 0
                for t in avail:
                    for _ in range(per):
                        if i < NBV:
                            sched[t].append((0, i))
                            i += 1
                early_done = i

            # ---- phase 2: recurrence ----
            def h_src(tt, k):
                if tt == 0:
                    return h0b[:, k * bl:(k + 1) * bl]
                row = (tt - 1) * bl
                for mi, (off, pm) in enumerate(mtiles):
                    if row < off + pm:
                        return hs_m[mi][:, k, row - off:row - off + bl]
                raise AssertionError

            h_old = h_prev_f32
            if do_gru:
                for t in range(t_steps):
                    ph = gpsum.tile([128, MT * bl], F32, tag="ph", name="ph")
                    for m in range(MT):
                        for k in range(KC):
                            nc.tensor.matmul(ph[:, m * bl:(m + 1) * bl],
                                             whh[:, k, m * 128:(m + 1) * 128],
                                             h_src(t, k), start=(k == 0),
                                             stop=(k == KC - 1))
                    RZ = 2 * KC * bl
                    NL = KC * bl
                    rz_pre = work.tile([128, RZ], F32, tag="rz_pre", name="rz_pre")
                    nc.vector.tensor_tensor(rz_pre,
                                            giT[:, 0:2 * KC, t * bl:(t + 1) * bl],
                                            ph[:, 0:RZ], op=ALU.add)
                    rz = work.tile([128, RZ], F32, tag="rz", name="rz")
                    nc.scalar.activation(rz, rz_pre, AF.Tanh, scale=0.5)
                    nc.vector.tensor_scalar(rz, rz, 0.5, 0.5, op0=ALU.mult,
                                            op1=ALU.add)
                    t1 = work.tile([128, NL], F32, tag="t1", name="t1")
                    nc.vector.tensor_tensor(t1, ph[:, RZ:RZ + NL], bhn, op=ALU.add)
                    t2 = work.tile([128, NL], F32, tag="t2", name="t2")
                    nc.vector.tensor_mul(t2, rz[:, 0:NL], t1)
                    n_pre = work.tile([128, NL], F32, tag="n_pre", name="n_pre")
                    nc.vector.tensor_tensor(n_pre,
                                            giT[:, 2 * KC:3 * KC, t * bl:(t + 1) * bl],
                                            t2, op=ALU.add)
                    n_t = work.tile([128, NL], F32, tag="n_t", name="n_t")
                    nc.scalar.activation(n_t, n_pre, AF.Tanh)
                    d_t = work.tile([128, NL], F32, tag="d_t", name="d_t")
                    nc.vector.tensor_sub(d_t, h_old, n_t)
                    hz = work.tile([128, NL], F32, tag="hz", name="hz")
                    nc.vector.tensor_mul(hz, rz[:, NL:RZ], d_t)
                    h_new = work.tile([128, NL], F32, tag="h_new", name="h_new")
                    nc.vector.tensor_add(h_new, n_t, hz)
                    row = t * bl
                    for mi, (off, pm) in enumerate(mtiles):
                        if row < off + pm:
                            nc.vector.tensor_copy(
                                hs_m[mi][:, :, row - off:row - off + bl],
                                h_new.rearrange("p (k b) -> p k b", b=bl))
                            break
                    h_old = h_new
                    for (mi, vb) in sched[t]:
                        wt, bt = stage_w(vb)
                        proj_block(mi, vb, wt, bt)

            nc.sync.dma_start(out=hf_d.ap(), in_=h_old)

            # ---- phase 3: remaining projection + softmax ----
            if do_proj:
                for vb in range(early_done, NBV):
                    wt, bt = stage_w(vb)
                    for mi in range(NMT):
                        proj_block(mi, vb, wt, bt)
                stats(0)
                ci = 0
                for vb in range(0, early_done):
                    wt, bt = stage_w(vb)
                    for mi in range(1, NMT):
                        proj_block(mi, vb, wt, bt)
                    if ci < NBV:
                        pass_c(0, ci)
                        ci += 1
                for mi in range(1, NMT):
                    stats(mi)
                while ci < NBV:
                    pass_c(0, ci)
                    ci += 1
                for mi in range(1, NMT):
                    for vb in range(NBV):
                        pass_c(mi, vb)

    nc.compile()
    return nc


# ---------------------------------------------------------------------------
# host-side sharding / marshalling
# ---------------------------------------------------------------------------

def _prep_core_inputs(c, E, h0, wihT_bf, whhT_bf, bgi, bhn, woutT_bf, bout_bf,
                      t_steps=T, h=H, bl=BL):
    KC = h // 128
    R = t_steps * bl
    Ec = E[c * bl:(c + 1) * bl]                       # [bl, T, H]
    xT = np.ascontiguousarray(Ec.transpose(2, 1, 0)).reshape(h, R)
    h0c = h0[c * bl:(c + 1) * bl]                     # [bl, H]
    h0T = np.ascontiguousarray(h0c.reshape(bl, KC, 128).transpose(2, 1, 0)
                               ).reshape(128, KC * bl)
    return {
        "xT": xT.astype(ml_dtypes.bfloat16),
        "h0T": h0T.astype(np.float32),
        "wihT": wihT_bf,
        "whhT": whhT_bf,
        "bgi": bgi,
        "bhn": bhn,
        "woutT": woutT_bf,
        "bout": bout_bf,
    }


def host_prep(encoder_outputs, encoder_hidden, target_tensor, embedding,
              W_ih, W_hh, b_ih, b_hh, W_out, b_out):
    target_tensor = np.asarray(target_tensor)
    embedding = np.asarray(embedding, dtype=np.float32)
    tokens = np.concatenate(
        [np.full((B, 1), SOS, dtype=target_tensor.dtype),
         target_tensor[:, :T - 1]], axis=1)            # [B, T]
    E = embedding[tokens]                              # [B, T, H] fp32
    h0 = np.asarray(encoder_hidden, dtype=np.float32)[0]  # [B, H]
    W_ih = np.asarray(W_ih, dtype=np.float32)
    W_hh = np.asarray(W_hh, dtype=np.float32)
    b_ih = np.asarray(b_ih, dtype=np.float32)
    b_hh = np.asarray(b_hh, dtype=np.float32)
    W_out = np.asarray(W_out, dtype=np.float32)
    b_out = np.asarray(b_out, dtype=np.float32)

    wihT_bf = np.ascontiguousarray(W_ih.T).astype(ml_dtypes.bfloat16)
    whhT_bf = np.ascontiguousarray(W_hh.T).astype(ml_dtypes.bfloat16)
    woutT_bf = np.ascontiguousarray(W_out.T).astype(ml_dtypes.bfloat16)
    bgi = b_ih.copy()
    bgi[:2 * H] += b_hh[:2 * H]
    KC = H // 128
    bhn = np.repeat(b_hh[2 * H:].reshape(KC, 128).T[:, :, None], BL, axis=2
                    ).reshape(128, KC * BL).astype(np.float32)
    bout_bf = b_out.astype(ml_dtypes.bfloat16)

    return [_prep_core_inputs(c, E, h0, wihT_bf, whhT_bf, bgi, bhn,
                              woutT_bf, bout_bf) for c in range(NCORES)]


def assemble_outputs(results):
    KC = H // 128
    lps = []
    hfs = []
    for c in range(NCORES):
        lp = results[c]["log_probs"].reshape(T, BL, V).transpose(1, 0, 2)
        lps.append(lp)
        hfT = results[c]["h_final"].reshape(128, KC, BL).transpose(2, 1, 0)
        hfs.append(hfT.reshape(BL, H))
    log_probs = np.concatenate(lps, axis=0)            # [B, T, V]
    h_final = np.concatenate(hfs, axis=0)[None]        # [1, B, H]
    return log_probs.astype(np.float32), h_final.astype(np.float32)


_NC_CACHE = {}


def kernel(encoder_outputs, encoder_hidden, target_tensor, embedding,
           W_ih, W_hh, b_ih, b_hh, W_out, b_out):
    in_maps = host_prep(encoder_outputs, encoder_hidden, target_tensor,
                        embedding, W_ih, W_hh, b_ih, b_hh, W_out, b_out)
    if "nc" not in _NC_CACHE:
        _NC_CACHE["nc"] = build_nc()
    nc = _NC_CACHE["nc"]
    res = run_bass_kernel_spmd(nc, in_maps, core_ids=list(range(NCORES)))
    return assemble_outputs(res.results)


# revision 10
# speedup vs baseline: 2.8800x; 2.8800x over previous
"""Trainium2 Bass kernel for nn_DecoderRNN: GRU decoder (teacher forcing) +
vocab projection + log_softmax.

Strategy: data-parallel over batch across 8 NeuronCores (8 batch rows/core).
Per core (all in "transposed" layouts, feature dims on SBUF partitions):
  - gi = relu(emb) @ W_ih.T + bias  precomputed for all 30 steps (one matmul)
  - 30-step GRU recurrence: ghT via weight-stationary bf16 matmuls,
    gates on [128, .] tiles, sigmoid computed via tanh (one ACT table set)
  - vocab projection (hs stationary, W_out.T streamed, b_out via K=1 matmul),
    interleaved into recurrence gaps once row-tile 0 is complete
  - two-pass log_softmax: per-block max/sum-exp stats during projection,
    bf16 logits round-trip through HBM, final per-partition lse subtract
"""

from contextlib import ExitStack, nullcontext

import numpy as np
import ml_dtypes

import concourse.bass as bass
import concourse.mybir as mybir
import concourse.tile as tile
from concourse import bacc
from concourse.bass_utils import run_bass_kernel_spmd

BF16 = mybir.dt.bfloat16
F32 = mybir.dt.float32
AF = mybir.ActivationFunctionType
ALU = mybir.AluOpType
AX = mybir.AxisListType

# problem dims (hardcoded per harness contract)
B = 64
T = 30
H = 1024
V = 32000
NCORES = 8
BL = B // NCORES          # batch rows per core
SOS = 1


def _blocks(total, step):
    out = []
    off = 0
    while off < total:
        sz = min(step, total - off)
        out.append((off, sz))
        off += sz
    return out


def build_nc(t_steps=T, vocab=V, h=H, bl=BL, num_devices=NCORES, nrep=1,
             mode="all"):
    """Build the per-core bass program (same program on all cores, SPMD).

    mode: 'all' | 'gru' (loads+gi+recurrence only) | 'proj' (projection only)
    nrep: repeat whole body in a hardware For_i loop (for wall-delta timing)
    """
    do_gru = mode in ("all", "gru", "all_noint")
    do_proj = mode in ("all", "proj", "all_noint", "dma", "projmm")
    skip_mm = mode == "dma"
    skip_soft = mode in ("dma", "projmm")
    KC = h // 128             # H chunks on partitions
    G3 = 3 * h
    MT = G3 // 128            # gate-dim tiles (r, z, n each KC tiles)
    R = t_steps * bl          # rows per core; row index = t*bl + b
    vblocks = _blocks(vocab, 512)
    mtiles = _blocks(R, 128)
    NBV = len(vblocks)
    NMT = len(mtiles)
    mtile_ready = [(off + pm - 1) // bl for off, pm in mtiles]

    nc = bacc.Bacc("TRN2", target_bir_lowering=False, debug=False,
                   num_devices=num_devices)

    # ---- DRAM I/O ----
    xT_d = nc.dram_tensor("xT", [h, R], BF16, kind="ExternalInput")
    h0_d = nc.dram_tensor("h0T", [128, KC * bl], F32, kind="ExternalInput")
    wih_d = nc.dram_tensor("wihT", [h, G3], BF16, kind="ExternalInput")
    whh_d = nc.dram_tensor("whhT", [h, G3], BF16, kind="ExternalInput")
    bgi_d = nc.dram_tensor("bgi", [G3], F32, kind="ExternalInput")
    bhn_d = nc.dram_tensor("bhn", [128, KC * bl], F32, kind="ExternalInput")
    wout_d = nc.dram_tensor("woutT", [h, vocab], BF16, kind="ExternalInput")
    bout_d = nc.dram_tensor("bout", [vocab], BF16, kind="ExternalInput")
    lp_d = nc.dram_tensor("log_probs", [R, vocab], F32, kind="ExternalOutput")
    hf_d = nc.dram_tensor("h_final", [128, KC * bl], F32, kind="ExternalOutput")
    scr_d = nc.dram_tensor("scratch", [R, vocab], BF16)  # internal HBM scratch

    with tile.TileContext(nc) as tc, ExitStack() as ctx:
        const = ctx.enter_context(tc.tile_pool(name="const", bufs=1))
        work = ctx.enter_context(tc.tile_pool(name="work", bufs=3))
        gpsum = ctx.enter_context(tc.tile_pool(name="gpsum", bufs=2, space="PSUM"))

        loop_cm = tc.For_i(0, nrep, 1) if nrep > 1 else nullcontext()
        with loop_cm:
            # ---- resident SBUF tiles ----
            whh = const.tile([128, KC, G3], BF16)
            xT = const.tile([128, KC, R], BF16)
            giT = const.tile([128, MT, R], F32)
            hs_m = [const.tile([128, KC, pm], BF16, name=f"hs{i}", tag=f"hs{i}")
                    for i, (off, pm) in enumerate(mtiles)]
            h0b = const.tile([128, KC * bl], BF16)
            bgi = const.tile([128, MT], F32)
            bhn = const.tile([128, KC * bl], F32)
            ones = const.tile([1, 128], BF16)
            nbm = [const.tile([128, NBV], F32, name=f"nbm{i}", tag=f"nbm{i}")
                   for i in range(NMT)]
            bsum = [const.tile([128, NBV], F32, name=f"bsum{i}", tag=f"bsum{i}")
                    for i in range(NMT)]
            nlse = [const.tile([128, 1], F32, name=f"nlse{i}", tag=f"nlse{i}")
                    for i in range(NMT)]

            # ---- load ----
            nc.vector.memset(ones, 1.0)
            h_prev_f32 = const.tile([128, KC * bl], F32, tag="h0f", name="h0f")
            nc.sync.dma_start(out=h_prev_f32, in_=h0_d.ap())
            if do_gru:
                nc.sync.dma_start(out=whh, in_=whh_d.ap().rearrange("(k p) m -> p k m", p=128))
                nc.sync.dma_start(out=xT, in_=xT_d.ap().rearrange("(k p) r -> p k r", p=128))
                nc.sync.dma_start(out=bgi, in_=bgi_d.ap().rearrange("(m p) -> p m", p=128))
                nc.sync.dma_start(out=bhn, in_=bhn_d.ap())
                nc.vector.tensor_relu(xT, xT)
                nc.vector.tensor_copy(h0b, h_prev_f32)
            else:
                nc.vector.memset(giT, 0.01)
                for mi in range(NMT):
                    nc.vector.memset(hs_m[mi], 0.01)

            # ---- phase 1: giT = W_ih @ x + bias (transposed layout) ----
            if do_gru:
                with (
                    tc.tile_pool(name="wihp", bufs=1) as wihp,
                    tc.tile_pool(name="gipsum", bufs=3, space="PSUM") as gipsum,
                ):
                    wih = wihp.tile([128, KC, G3], BF16)
                    nc.sync.dma_start(out=wih, in_=wih_d.ap().rearrange("(k p) m -> p k m", p=128))
                    for m in range(MT):
                        pg = gipsum.tile([128, R], F32, tag="pg", name="pg")
                        for k in range(KC):
                            nc.tensor.matmul(pg, wih[:, k, m * 128:(m + 1) * 128],
                                             xT[:, k, :], start=(k == 0),
                                             stop=(k == KC - 1))
                        nc.vector.tensor_scalar(giT[:, m, :], pg, bgi[:, m:m + 1],
                                                None, op0=ALU.add)

            # ---- projection / softmax helpers ----
            wpool = ctx.enter_context(tc.tile_pool(name="wstage", bufs=4))
            lgpool = ctx.enter_context(tc.tile_pool(name="lg", bufs=4))
            outp = ctx.enter_context(tc.tile_pool(name="outp", bufs=4))
            ppsum = ctx.enter_context(tc.tile_pool(name="ppsum", bufs=3, space="PSUM"))

            def stage_w(vb):
                voff, vn = vblocks[vb]
                wt = wpool.tile([128, KC, 512], BF16, tag="wt", name="wt")
                nc.sync.dma_start(
                    out=wt[:, :, :vn],
                    in_=wout_d.ap()[:, voff:voff + vn].rearrange("(k p) v -> p k v", p=128))
                bt = wpool.tile([1, 512], BF16, tag="bt", name="bt")
                nc.sync.dma_start(out=bt[:, :vn],
                                  in_=bout_d.ap()[voff:voff + vn].rearrange("(a v) -> a v", a=1))
                return wt, bt

            def proj_block(mi, vb, wt, bt):
                moff, pm = mtiles[mi]
                voff, vn = vblocks[vb]
                if skip_mm:
                    return
                pl = ppsum.tile([128, 512], F32, tag="pl", name="pl")
                nc.tensor.matmul(pl[:pm, :vn], ones[:, :pm], bt[:, :vn],
                                 start=True, stop=False)
                for k in range(KC):
                    nc.tensor.matmul(pl[:pm, :vn], hs_m[mi][:, k, :pm], wt[:, k, :vn],
                                     start=False, stop=(k == KC - 1))
                if skip_soft:
                    lg = lgpool.tile([128, 512], BF16, tag="lg", name="lg")
                    nc.scalar.copy(lg[:pm, :vn], pl[:pm, :vn])
                    nc.sync.dma_start(out=scr_d.ap()[moff:moff + pm, voff:voff + vn],
                                      in_=lg[:pm, :vn])
                    return
                nc.vector.tensor_reduce(nbm[mi][:pm, vb:vb + 1], pl[:pm, :vn],
                                        axis=AX.X, op=ALU.max, negate=True)
                lg = lgpool.tile([128, 512], BF16, tag="lg", name="lg")
                nc.scalar.copy(lg[:pm, :vn], pl[:pm, :vn])
                nc.sync.dma_start(out=scr_d.ap()[moff:moff + pm, voff:voff + vn],
                                  in_=lg[:pm, :vn])
                et = lgpool.tile([128, 512], BF16, tag="et", name="et")
                nc.scalar.activation(et[:pm, :vn], pl[:pm, :vn], AF.Exp,
                                     bias=nbm[mi][:pm, vb:vb + 1],
                                     accum_out=bsum[mi][:pm, vb:vb + 1])

            def stats(mi):
                moff, pm = mtiles[mi]
                negM = work.tile([128, 1], F32, tag="negM", name="negM")
                nc.vector.tensor_reduce(negM[:pm], nbm[mi][:pm, :], axis=AX.X,
                                        op=ALU.min)
                corr = work.tile([128, NBV], F32, tag="corr", name="corr")
                nc.scalar.activation(corr[:pm], nbm[mi][:pm, :], AF.Exp,
                                     bias=negM[:pm], scale=-1.0)
                prod = work.tile([128, NBV], F32, tag="prod", name="prod")
                nc.vector.tensor_mul(prod[:pm], corr[:pm], bsum[mi][:pm, :])
                S = work.tile([128, 1], F32, tag="S", name="S")
                nc.vector.tensor_reduce(S[:pm], prod[:pm], axis=AX.X, op=ALU.add)
                lnS = work.tile([128, 1], F32, tag="lnS", name="lnS")
                nc.scalar.activation(lnS[:pm], S[:pm], AF.Ln)
                nc.vector.tensor_tensor(nlse[mi][:pm], negM[:pm], lnS[:pm],
                                        op=ALU.subtract)

            def pass_c(mi, vb):
                moff, pm = mtiles[mi]
                voff, vn = vblocks[vb]
                lr = lgpool.tile([128, 512], BF16, tag="lr", name="lr")
                nc.sync.dma_start(out=lr[:pm, :vn],
                                  in_=scr_d.ap()[moff:moff + pm, voff:voff + vn])
                of = outp.tile([128, 512], F32, tag="of", name="of")
                nc.vector.tensor_scalar(of[:pm, :vn], lr[:pm, :vn],
                                        nlse[mi][:pm], None, op0=ALU.add)
                nc.sync.dma_start(out=lp_d.ap()[moff:moff + pm, voff:voff + vn],
                                  in_=of[:pm, :vn])

            # schedule: row-tile 0 blocks spread over steps after it completes
            sched = {t: [] for t in range(t_steps)}
            early_done = 0
            if (mode != "all_noint") and do_proj and do_gru and NMT > 1 and mtile_ready[0] < t_steps - 1:
                avail = list(range(mtile_ready[0] + 1, t_steps))
                per = (NBV + len(avail) - 1) // len(avail)
                i = 0
                for t in avail:
                    for _ in range(per):
                        if i < NBV:
                            sched[t].append((0, i))
                            i += 1
                early_done = i

            # ---- phase 2: recurrence ----
            def h_src(tt, k):
                if tt == 0:
                    return h0b[:, k * bl:(k + 1) * bl]
                row = (tt - 1) * bl
                for mi, (off, pm) in enumerate(mtiles):
                    if row < off + pm:
                        return hs_m[mi][:, k, row - off:row - off + bl]
                raise AssertionError

            h_old = h_prev_f32
            if do_gru:
                for t in range(t_steps):
                    ph = gpsum.tile([128, MT * bl], F32, tag="ph", name="ph")
                    for m in range(MT):
                        for k in range(KC):
                            nc.tensor.matmul(ph[:, m * bl:(m + 1) * bl],
                                             whh[:, k, m * 128:(m + 1) * 128],
                                             h_src(t, k), start=(k == 0),
                                             stop=(k == KC - 1))
                    RZ = 2 * KC * bl
                    NL = KC * bl
                    rz_pre = work.tile([128, RZ], F32, tag="rz_pre", name="rz_pre")
                    nc.vector.tensor_tensor(rz_pre,
                                            giT[:, 0:2 * KC, t * bl:(t + 1) * bl],
                                            ph[:, 0:RZ], op=ALU.add)
                    rz = work.tile([128, RZ], F32, tag="rz", name="rz")
                    nc.scalar.activation(rz, rz_pre, AF.Tanh, scale=0.5)
                    nc.vector.tensor_scalar(rz, rz, 0.5, 0.5, op0=ALU.mult,
                                            op1=ALU.add)
                    t1 = work.tile([128, NL], F32, tag="t1", name="t1")
                    nc.vector.tensor_tensor(t1, ph[:, RZ:RZ + NL], bhn, op=ALU.add)
                    t2 = work.tile([128, NL], F32, tag="t2", name="t2")
                    nc.vector.tensor_mul(t2, rz[:, 0:NL], t1)
                    n_pre = work.tile([128, NL], F32, tag="n_pre", name="n_pre")
                    nc.vector.tensor_tensor(n_pre,
                                            giT[:, 2 * KC:3 * KC, t * bl:(t + 1) * bl],
                                            t2, op=ALU.add)
                    n_t = work.tile([128, NL], F32, tag="n_t", name="n_t")
                    nc.scalar.activation(n_t, n_pre, AF.Tanh)
                    d_t = work.tile([128, NL], F32, tag="d_t", name="d_t")
                    nc.vector.tensor_sub(d_t, h_old, n_t)
                    hz = work.tile([128, NL], F32, tag="hz", name="hz")
                    nc.vector.tensor_mul(hz, rz[:, NL:RZ], d_t)
                    h_new = work.tile([128, NL], F32, tag="h_new", name="h_new")
                    nc.vector.tensor_add(h_new, n_t, hz)
                    row = t * bl
                    for mi, (off, pm) in enumerate(mtiles):
                        if row < off + pm:
                            nc.vector.tensor_copy(
                                hs_m[mi][:, :, row - off:row - off + bl],
                                h_new.rearrange("p (k b) -> p k b", b=bl))
                            break
                    h_old = h_new
                    for (mi, vb) in sched[t]:
                        wt, bt = stage_w(vb)
                        proj_block(mi, vb, wt, bt)

            nc.sync.dma_start(out=hf_d.ap(), in_=h_old)

            # ---- phase 3: remaining projection + softmax ----
            if do_proj:
                for vb in range(early_done, NBV):
                    wt, bt = stage_w(vb)
                    for mi in range(NMT):
                        proj_block(mi, vb, wt, bt)
                if skip_soft:
                    nc.vector.memset(nlse[0], 0.0)
                stats(0) if not skip_soft else None
                ci = 0
                for vb in range(0, early_done):
                    wt, bt = stage_w(vb)
                    for mi in range(1, NMT):
                        proj_block(mi, vb, wt, bt)
                    if ci < NBV and not skip_soft:
                        pass_c(0, ci)
                        ci += 1
                for mi in range(1, NMT):
                    if not skip_soft:
                        stats(mi)
                    else:
                        nc.vector.memset(nlse[mi], 0.0)
                if not skip_soft:
                    while ci < NBV:
                        pass_c(0, ci)
                        ci += 1
                    for mi in range(1, NMT):
                        for vb in range(NBV):
                            pass_c(mi, vb)

    nc.compile()
    return nc


# ---------------------------------------------------------------------------
# host-side sharding / marshalling
# ---------------------------------------------------------------------------

def _prep_core_inputs(c, E, h0, wihT_bf, whhT_bf, bgi, bhn, woutT_bf, bout_bf,
                      t_steps=T, h=H, bl=BL):
    KC = h // 128
    R = t_steps * bl
    Ec = E[c * bl:(c + 1) * bl]                       # [bl, T, H]
    xT = np.ascontiguousarray(Ec.transpose(2, 1, 0)).reshape(h, R)
    h0c = h0[c * bl:(c + 1) * bl]                     # [bl, H]
    h0T = np.ascontiguousarray(h0c.reshape(bl, KC, 128).transpose(2, 1, 0)
                               ).reshape(128, KC * bl)
    return {
        "xT": xT.astype(ml_dtypes.bfloat16),
        "h0T": h0T.astype(np.float32),
        "wihT": wihT_bf,
        "whhT": whhT_bf,
        "bgi": bgi,
        "bhn": bhn,
        "woutT": woutT_bf,
        "bout": bout_bf,
    }


def host_prep(encoder_outputs, encoder_hidden, target_tensor, embedding,
              W_ih, W_hh, b_ih, b_hh, W_out, b_out):
    target_tensor = np.asarray(target_tensor)
    embedding = np.asarray(embedding, dtype=np.float32)
    tokens = np.concatenate(
        [np.full((B, 1), SOS, dtype=target_tensor.dtype),
         target_tensor[:, :T - 1]], axis=1)            # [B, T]
    E = embedding[tokens]                              # [B, T, H] fp32
    h0 = np.asarray(encoder_hidden, dtype=np.float32)[0]  # [B, H]
    W_ih = np.asarray(W_ih, dtype=np.float32)
    W_hh = np.asarray(W_hh, dtype=np.float32)
    b_ih = np.asarray(b_ih, dtype=np.float32)
    b_hh = np.asarray(b_hh, dtype=np.float32)
    W_out = np.asarray(W_out, dtype=np.float32)
    b_out = np.asarray(b_out, dtype=np.float32)

    wihT_bf = np.ascontiguousarray(W_ih.T).astype(ml_dtypes.bfloat16)
    whhT_bf = np.ascontiguousarray(W_hh.T).astype(ml_dtypes.bfloat16)
    woutT_bf = np.ascontiguousarray(W_out.T).astype(ml_dtypes.bfloat16)
    bgi = b_ih.copy()
    bgi[:2 * H] += b_hh[:2 * H]
    KC = H // 128
    bhn = np.repeat(b_hh[2 * H:].reshape(KC, 128).T[:, :, None], BL, axis=2
                    ).reshape(128, KC * BL).astype(np.float32)
    bout_bf = b_out.astype(ml_dtypes.bfloat16)

    return [_prep_core_inputs(c, E, h0, wihT_bf, whhT_bf, bgi, bhn,
                              woutT_bf, bout_bf) for c in range(NCORES)]


def assemble_outputs(results):
    KC = H // 128
    lps = []
    hfs = []
    for c in range(NCORES):
        lp = results[c]["log_probs"].reshape(T, BL, V).transpose(1, 0, 2)
        lps.append(lp)
        hfT = results[c]["h_final"].reshape(128, KC, BL).transpose(2, 1, 0)
        hfs.append(hfT.reshape(BL, H))
    log_probs = np.concatenate(lps, axis=0)            # [B, T, V]
    h_final = np.concatenate(hfs, axis=0)[None]        # [1, B, H]
    return log_probs.astype(np.float32), h_final.astype(np.float32)


_NC_CACHE = {}


def kernel(encoder_outputs, encoder_hidden, target_tensor, embedding,
           W_ih, W_hh, b_ih, b_hh, W_out, b_out):
    in_maps = host_prep(encoder_outputs, encoder_hidden, target_tensor,
                        embedding, W_ih, W_hh, b_ih, b_hh, W_out, b_out)
    if "nc" not in _NC_CACHE:
        _NC_CACHE["nc"] = build_nc()
    nc = _NC_CACHE["nc"]
    res = run_bass_kernel_spmd(nc, in_maps, core_ids=list(range(NCORES)))
    return assemble_outputs(res.results)
